# revision 40
# baseline (speedup 1.0000x reference)
"""AverageAttention Trainium2 kernel (v3).

Computes, per batch b (data-parallel across 8 NeuronCores):
    avg      = cumsum(x, axis=seq) / (pos+1)
    inter    = relu(LN(avg) @ w1 + b1)
    avg_out  = inter @ w2 + b2 + avg
    gates    = [x, avg_out] @ wg + bg
    gated    = sigmoid(gates[:, :D]) * x + sigmoid(gates[:, D:]) * avg_out
returns (gated, avg_out), each [B, S, D].

Structure:
  Phase A (~13us, DMA-bound): stream x (bf16); per 128-row block two
    E_i-masked matmuls accumulate the block totals into [16, 512]x2
    PSUM. One strict-upper [16,16] tri matmul then yields every block
    prefix at once -- no serial carry chain. The per-block carry later
    reads prefix row i straight out of the [16, D] prefix tile through a
    row-selector matmul (lhsT = esel[:, i, :]), so no partition
    scatter / extra DMAs are needed.
  Phase B (~125us, PE-bound): per block, software-pipelined: in-block
    cumsum via bf16 tri matmul + prefix-row carry, LN (affine folded
    into w1/b1 on host), FFN1/FFN2 in bf16, residual; avg_out streams
    to DRAM and stays SBUF-resident for phase C. ALL transposes (x, z,
    avg_out) ride the XBAR dma-transpose so the PE does only matmuls;
    fp8 casts for the gating operands happen on ScalarE copies.
  Phase C (~57us): gating matmuls in fp8 e4m3 DoubleRow mode (K=256
    per matmul at 0.5 cyc/row). wg arrives host-scaled by 256, host
    pre-tiled to the SBUF layout so each quarter is ONE contiguous
    128-descriptor DMA. The descale rides the sigmoid activation
    scale; gated multiplies use exact fp32 x (re-DMA) and the
    SBUF-resident fp32 avg_out, all on DVE (Pool only does the
    bf16 staging cast in B).
"""

import os
import sys

if "/opt/trn_rl_repo" not in sys.path:
    sys.path.insert(0, "/opt/trn_rl_repo")

# The NEFF executes via the axon-tunneled PJRT backend; a JAX_PLATFORMS=cpu
# pin (used for running references) would hide the NeuronCores.
if os.environ.get("JAX_PLATFORMS") == "cpu":
    os.environ.pop("JAX_PLATFORMS")

from contextlib import ExitStack

import ml_dtypes
import numpy as np

import concourse.bass as bass
import concourse.mybir as mybir
import concourse.tile as tile
from concourse import bacc
from concourse.bass_utils import run_bass_kernel_spmd

B, S, D = 8, 2048, 1024
P = 128
NBLK = S // P            # 16 seq blocks per core
D2 = 2 * D
KC = D // P              # 8 feature chunks of 128
EPS = 1e-6
WG_SCALE = 256.0         # host-folded into wg8*; descaled in the sigmoid

FP32 = mybir.dt.float32
BF16 = mybir.dt.bfloat16
F32R = mybir.dt.float32r
FP8 = mybir.dt.float8e4

AF = mybir.ActivationFunctionType
ALU = mybir.AluOpType
DR = mybir.MatmulPerfMode.DoubleRow


def build_program(has_b2: bool, has_bg: bool) -> bacc.Bacc:
    nc = bacc.Bacc("TRN2", target_bir_lowering=False, debug=False, num_devices=8)

    x_d = nc.declare_dram_parameter("x", [S, D], FP32, isOutput=False)
    xbf_d = nc.declare_dram_parameter("xbf", [S, D], BF16, isOutput=False)
    w1_d = nc.declare_dram_parameter("w1g", [D, D], BF16, isOutput=False)
    b1_d = nc.declare_dram_parameter("b1p", [D], FP32, isOutput=False)
    w2_d = nc.declare_dram_parameter("w2", [D, D], BF16, isOutput=False)
    # gating weights, host-tiled to [part, kchunk, 512] so each quarter is
    # one contiguous 128-descriptor DMA
    wgq_d = {
        q: nc.declare_dram_parameter(f"wg8{q}", [P, 2 * KC, 512], FP8, isOutput=False)
        for q in ("in0", "fg0", "in1", "fg1")
    }
    tri_d = nc.declare_dram_parameter("tri", [P, P], BF16, isOutput=False)
    tri16_d = nc.declare_dram_parameter("tri16", [16, 16], BF16, isOutput=False)
    eblk_d = nc.declare_dram_parameter("eblk", [P, NBLK, 16], BF16, isOutput=False)
    esel_d = nc.declare_dram_parameter("esel", [16, NBLK, P], BF16, isOutput=False)
    inv_d = nc.declare_dram_parameter("invpos", [P, NBLK], FP32, isOutput=False)
    if has_b2:
        b2_d = nc.declare_dram_parameter("b2", [D], FP32, isOutput=False)
    if has_bg:
        bg_d = nc.declare_dram_parameter("bg", [D2], FP32, isOutput=False)

    gated_d = nc.declare_dram_parameter("gated", [S, D], FP32, isOutput=True)
    aout_d = nc.declare_dram_parameter("avg_out", [S, D], FP32, isOutput=True)

    xbf_r = xbf_d[:].rearrange("(n p) d -> p n d", p=P)    # [128, 16, 1024]
    aout_r = aout_d[:].rearrange("(n p) d -> p n d", p=P)
    w1_r = w1_d[:].rearrange("(c p) f -> p c f", p=P)      # [128, 8, 1024]
    w2_r = w2_d[:].rearrange("(c p) f -> p c f", p=P)

    with tile.TileContext(nc) as tc, ExitStack() as ctx:
        const = ctx.enter_context(tc.tile_pool(name="const", bufs=1))

        xT8 = const.tile([P, KC, S], FP8)       # x^T e4m3, gating lhsT
        aoT8 = const.tile([P, KC, S], FP8)      # avg_out^T e4m3
        ao_sb = const.tile([P, NBLK, D], FP32)  # avg_out resident for phase C
        totals16 = const.tile([16, D], BF16)
        prefix16 = const.tile([16, D], BF16)

        eblk_sb = const.tile([P, NBLK, 16], BF16)
        nc.sync.dma_start(out=eblk_sb, in_=eblk_d[:])
        inv_sb = const.tile([P, NBLK], FP32)
        b1t_sb = const.tile([P, KC], FP32)
        tri_sb = const.tile([P, P], BF16)
        tri16_sb = const.tile([16, 16], BF16)
        esel_sb = const.tile([16, NBLK, P], BF16)

        def load_late_consts():
            # everything here is first consumed in late phase A or phase B
            nc.sync.dma_start(out=inv_sb, in_=inv_d[:])
            nc.sync.dma_start(out=b1t_sb, in_=b1_d[:].rearrange("(c p) -> p c", p=P))
            nc.sync.dma_start(out=tri_sb, in_=tri_d[:])
            nc.sync.dma_start(out=tri16_sb, in_=tri16_d[:])
            nc.sync.dma_start(out=esel_sb, in_=esel_d[:])
        # int32 seed constant for the DVE fast-inverse-sqrt (keeps Sqrt off
        # ScalarE so the whole kernel fits one ACT table set)
        magic_sb = const.tile([P, 1], mybir.dt.int32)
        nc.vector.memset(magic_sb, 0x5F3759DF)
        if has_b2:
            b2r_sb = const.tile([P, D], FP32)
            nc.sync.dma_start(out=b2r_sb, in_=b2_d[None, :].to_broadcast([P, D]))
        if has_bg:
            bgr_sb = const.tile([P, D2], FP32)
            nc.sync.dma_start(out=bgr_sb, in_=bg_d[None, :].to_broadcast([P, D2]))

        wgpre_p = ctx.enter_context(tc.tile_pool(name="wgpre", bufs=1))
        wg_in0 = wgpre_p.tile([P, 2 * KC, 512], FP8)   # dh=0 gating weights
        wg_fg0 = wgpre_p.tile([P, 2 * KC, 512], FP8)

        ctxW = ExitStack()
        w12 = ctxW.enter_context(tc.tile_pool(name="w12", bufs=1))
        w1_sb = w12.tile([P, KC, D], BF16)
        w2_sb = w12.tile([P, KC, D], BF16)

        # ---------------- Phase A: block totals + prefix --------------------
        with ExitStack() as ctxA:
            tot_ps = ctxA.enter_context(
                tc.tile_pool(name="tot_ps", bufs=1, space="PSUM")
            )
            xa_p = ctxA.enter_context(tc.tile_pool(name="xa", bufs=5))

            tot_psl = [
                tot_ps.tile([16, 512], FP32, tag=f"tot{c}", name=f"tot{c}")
                for c in range(2)
            ]
            xa_tiles = {}

            def issue_xa(i):
                if i >= NBLK:
                    return
                t = xa_p.tile([P, D], BF16, tag="xa")
                nc.sync.dma_start(out=t, in_=xbf_r[:, i, :])
                xa_tiles[i] = t

            issue_xa(0)
            issue_xa(1)
            load_late_consts()
            for i in range(NBLK):
                xa = xa_tiles.pop(i)
                issue_xa(i + 2)
                for c in range(2):
                    nc.tensor.matmul(
                        tot_psl[c],
                        lhsT=eblk_sb[:, i, :],
                        rhs=xa[:, c * 512 : (c + 1) * 512],
                        start=(i == 0), stop=(i == NBLK - 1),
                    )

            # block totals -> strict-prefix rows via one [16,16] tri matmul
            for c in range(2):
                nc.scalar.copy(
                    out=totals16[:, c * 512 : (c + 1) * 512], in_=tot_psl[c]
                )
            for c in range(2):
                pre_ps = tot_ps.tile([16, 512], FP32, tag=f"pre{c}")
                nc.tensor.matmul(
                    pre_ps, lhsT=tri16_sb,
                    rhs=totals16[:, c * 512 : (c + 1) * 512],
                    start=True, stop=True,
                )
                nc.scalar.copy(
                    out=prefix16[:, c * 512 : (c + 1) * 512], in_=pre_ps
                )

        mm_ps = ctx.enter_context(tc.tile_pool(name="mm_ps", bufs=5, space="PSUM"))

        # ---------------- Phase B: cumsum + LN + FFN, 1-block pipeline ------
        with ExitStack() as ctxB:
            f1_ps = ctxB.enter_context(
                tc.tile_pool(name="f1_ps", bufs=3, space="PSUM")
            )
            xb_p = ctxB.enter_context(tc.tile_pool(name="xb", bufs=4))
            avg_p = ctxB.enter_context(tc.tile_pool(name="avg", bufs=3))
            z_p = ctxB.enter_context(tc.tile_pool(name="z", bufs=2))
            zT_p = ctxB.enter_context(tc.tile_pool(name="zT", bufs=2))
            xT_p = ctxB.enter_context(tc.tile_pool(name="xT", bufs=2))
            intT_p = ctxB.enter_context(tc.tile_pool(name="intT", bufs=2))
            aobf_p = ctxB.enter_context(tc.tile_pool(name="aobf", bufs=2))
            stat_p = ctxB.enter_context(tc.tile_pool(name="stat", bufs=6))

            xb_tiles = {}
            avg_t = {}
            z_t = {}
            zT_t = {}
            xT_t = {}
            intT_t = {}
            aoT_t = {}
            aobf_t = {}

            def issue_xb(i):
                if i >= NBLK:
                    return
                t = xb_p.tile([P, D], BF16, tag="xb")
                nc.sync.dma_start(out=t, in_=xbf_r[:, i, :])
                xb_tiles[i] = t

            def s1_cumsum(i):
                """tri cumsum + prefix-row carry -> avg(i) (ACT evict w/ 1/pos)."""
                xb = xb_tiles[i]
                issue_xb(i + 2)
                avg_q = avg_p.tile([P, D], FP32, tag="avg")
                for c in range(2):
                    cs = slice(c * 512, (c + 1) * 512)
                    ps = mm_ps.tile([P, 512], FP32, tag="mm")
                    nc.tensor.matmul(
                        ps, lhsT=tri_sb, rhs=xb[:, cs],
                        start=True, stop=(i == 0),
                    )
                    if i > 0:
                        nc.tensor.matmul(
                            ps,
                            lhsT=esel_sb[:, i, :],
                            rhs=prefix16[:, cs],
                            start=False, stop=True,
                        )
                    nc.vector.tensor_scalar(
                        out=avg_q[:, cs], in0=ps,
                        scalar1=inv_sb[:, i : i + 1], scalar2=None, op0=ALU.mult,
                    )
                avg_t[i] = avg_q

            def s2_ln(i):
                """LayerNorm stats + normalize (gain/bias folded into w1)."""
                avg_q = avg_t[i]
                z_q = z_p.tile([P, D], BF16, tag="z")
                st = stat_p.tile([P, 2, 6], FP32, tag="st")
                for g in range(2):
                    nc.vector.bn_stats(
                        out=st[:, g, :], in_=avg_q[:, g * 512 : (g + 1) * 512]
                    )
                mv = stat_p.tile([P, 2], FP32, tag="mv")
                nc.vector.bn_aggr(out=mv, in_=st)
                y = stat_p.tile([P, 1], FP32, tag="y")
                nc.vector.tensor_scalar(
                    out=y, in0=mv[:, 1:2], scalar1=EPS, scalar2=None, op0=ALU.add
                )
                r0b = stat_p.tile([P, 1], mybir.dt.int32, tag="r0b")
                nc.vector.tensor_scalar(
                    out=r0b, in0=y[:].bitcast(mybir.dt.int32), scalar1=1,
                    scalar2=None, op0=ALU.logical_shift_right,
                )
                nc.vector.tensor_tensor(
                    out=r0b, in0=magic_sb, in1=r0b, op=ALU.subtract
                )
                rstd = r0b[:].bitcast(FP32)
                t = stat_p.tile([P, 1], FP32, tag="t")
                for _ in range(3):
                    nc.vector.tensor_tensor(out=t, in0=rstd, in1=rstd, op=ALU.mult)
                    nc.vector.tensor_tensor(out=t, in0=t, in1=y, op=ALU.mult)
                    nc.vector.tensor_scalar(
                        out=t, in0=t, scalar1=-0.5, scalar2=1.5,
                        op0=ALU.mult, op1=ALU.add,
                    )
                    nc.vector.tensor_tensor(out=rstd, in0=rstd, in1=t, op=ALU.mult)
                nc.vector.tensor_scalar(
                    out=z_q, in0=avg_q,
                    scalar1=mv[:, 0:1], scalar2=rstd,
                    op0=ALU.subtract, op1=ALU.mult,
                )
                if has_b2:
                    nc.gpsimd.tensor_add(out=avg_q, in0=avg_q, in1=b2r_sb)
                z_t[i] = z_q

            def s3x_trans(i):
                """x^T via XBAR dma-transpose (bf16 staging, cast next iter)."""
                xT_q = xT_p.tile([P, KC, P], BF16, tag="xT")
                nc.sync.dma_start_transpose(out=xT_q, in_=xb_tiles.pop(i)[:])
                xT_t[i] = xT_q

            def s3x_cast(i):
                nc.scalar.copy(
                    out=xT8[:, :, i * P : (i + 1) * P], in_=xT_t.pop(i)[:]
                )

            def s3z_trans(i):
                zT_q = zT_p.tile([P, KC, P], BF16, tag="zT")
                nc.sync.dma_start_transpose(out=zT_q, in_=z_t.pop(i)[:])
                zT_t[i] = zT_q

            def s4_ffn1(i):
                zT_q = zT_t.pop(i)
                intT_q = intT_p.tile([P, KC, P], BF16, tag="intT")
                for fc in range(KC):
                    ps = f1_ps.tile([P, P], FP32, tag="f1")
                    for k in range(KC):
                        nc.tensor.matmul(
                            ps,
                            lhsT=w1_sb[:, k, fc * P : (fc + 1) * P],
                            rhs=zT_q[:, k, :],
                            start=(k == 0), stop=(k == KC - 1),
                        )
                    nc.scalar.activation(
                        out=intT_q[:, fc, :], in_=ps,
                        func=AF.Relu, bias=b1t_sb[:, fc : fc + 1],
                    )
                intT_t[i] = intT_q

            def s5_ffn2(i):
                intT_q = intT_t.pop(i)
                avg_q = avg_t.pop(i)
                for dc in range(2):
                    ds_ = slice(dc * 512, (dc + 1) * 512)
                    ps = mm_ps.tile([P, 512], FP32, tag="mm")
                    for f in range(KC):
                        nc.tensor.matmul(
                            ps,
                            lhsT=intT_q[:, f, :],
                            rhs=w2_sb[:, f, ds_],
                            start=(f == 0), stop=(f == KC - 1),
                        )
                    nc.vector.tensor_add(
                        out=ao_sb[:, i, ds_], in0=ps, in1=avg_q[:, ds_]
                    )

            def s5w_aout(i):
                nc.sync.dma_start(out=aout_r[:, i, :], in_=ao_sb[:, i, :])

            def s6a_copy(i):
                """avg_out -> bf16 on Pool (halves: shorter engine holds)."""
                ao_bf = aobf_p.tile([P, D], BF16, tag="aobf")
                nc.gpsimd.tensor_copy(out=ao_bf[:, 0:512], in_=ao_sb[:, i, 0:512])
                nc.gpsimd.tensor_copy(out=ao_bf[:, 512:D], in_=ao_sb[:, i, 512:D])
                aobf_t[i] = ao_bf

            def s6a_dmat(i):
                """XBAR transpose one slot later: its input is long done, so
                the wait can't hold the DMA queue sequencer."""
                aoT_bf = aobf_p.tile([P, KC, P], BF16, tag="aoTbf")
                nc.sync.dma_start_transpose(out=aoT_bf, in_=aobf_t.pop(i)[:])
                aoT_t[i] = aoT_bf

            def s6b_aocast(i):
                nc.scalar.copy(
                    out=aoT8[:, :, i * P : (i + 1) * P], in_=aoT_t.pop(i)[:]
                )

            issue_xb(0)
            issue_xb(1)
            # FFN weights queue right behind the first two x tiles: w1 lands
            # just before the first FFN1, w2 just before the first FFN2.
            nc.sync.dma_start(out=w1_sb, in_=w1_r)
            nc.sync.dma_start(out=w2_sb, in_=w2_r)
            for it in range(NBLK + 5):
                if it < NBLK:
                    s1_cumsum(it)
                    s2_ln(it)
                    s3x_trans(it)
                if 1 <= it <= NBLK:
                    # one slot late: the transpose's wait on z is served while
                    # holding the queue SEQ, so same-slot emission head-blocks
                    s3z_trans(it - 1)
                    s3x_cast(it - 1)
                if 2 <= it <= NBLK + 1:
                    s6a_copy(it - 2)
                if 3 <= it <= NBLK + 2:
                    s6a_dmat(it - 3)
                if 1 <= it <= NBLK:
                    s4_ffn1(it - 1)
                    s5_ffn2(it - 1)
                if 2 <= it <= NBLK + 1:
                    s5w_aout(it - 2)
                if 4 <= it <= NBLK + 3:
                    s6b_aocast(it - 4)
                if it == NBLK - 4:
                    # prefetch the dh=0 gating weights (one DMA each)
                    nc.gpsimd.dma_start(out=wg_in0, in_=wgq_d["in0"][:])
                    nc.gpsimd.dma_start(out=wg_fg0, in_=wgq_d["fg0"][:])

        ctxW.close()  # w1/w2 are dead after phase B

        # ---------------- Phase C: fp8 DoubleRow gating ---------------------
        wg_p = ctx.enter_context(tc.tile_pool(name="wg", bufs=1))
        sig_p = ctx.enter_context(tc.tile_pool(name="sig", bufs=4))
        re_p = ctx.enter_context(tc.tile_pool(name="re", bufs=3))
        g_p = ctx.enter_context(tc.tile_pool(name="g", bufs=3))

        wg_in1 = wg_p.tile([P, 2 * KC, 512], FP8, tag="wgin1")
        wg_fg1 = wg_p.tile([P, 2 * KC, 512], FP8, tag="wgfg1")

        gt_prev = [None]
        for dh in range(2):  # output feature half (512 wide)
            ds_ = slice(dh * 512, (dh + 1) * 512)
            wg_in, wg_fg = (wg_in0, wg_fg0) if dh == 0 else (wg_in1, wg_fg1)
            for sb in range(NBLK):
                scol = slice(sb * P, (sb + 1) * P)
                if dh == 0 and sb == 1:
                    # prefetch the dh=1 gating weights during dh=0
                    nc.gpsimd.dma_start(out=wg_in1, in_=wgq_d["in1"][:])
                    nc.gpsimd.dma_start(out=wg_fg1, in_=wgq_d["fg1"][:])
                x_re = re_p.tile([P, 512], FP32, tag="xre")
                nc.sync.dma_start(out=x_re, in_=x_d[sb * P : (sb + 1) * P, ds_])
                ps_pair = []
                for wgt in (wg_in, wg_fg):
                    ps = mm_ps.tile([P, 512], FP32, tag="mm")
                    for kp in range(KC):  # K = 2048 as 8 pairs of 128
                        src = xT8 if kp < KC // 2 else aoT8
                        ko = 2 * kp if kp < KC // 2 else 2 * kp - KC
                        nc.tensor.matmul(
                            ps,
                            lhsT=src[:, ko : ko + 2, scol],
                            rhs=wgt[:, 2 * kp : 2 * kp + 2, :],
                            start=(kp == 0), stop=(kp == KC - 1),
                            perf_mode=DR,
                        )
                    ps_pair.append(ps)
                sig_in = sig_p.tile([P, 512], FP32, tag="sig")
                sig_fg = sig_p.tile([P, 512], FP32, tag="sig")
                if has_bg:
                    nc.vector.tensor_scalar(
                        out=sig_in, in0=ps_pair[0], scalar1=1.0 / WG_SCALE,
                        scalar2=None, op0=ALU.mult,
                    )
                    nc.vector.tensor_add(out=sig_in, in0=sig_in, in1=bgr_sb[:, ds_])
                    nc.scalar.activation(out=sig_in, in_=sig_in, func=AF.Sigmoid)
                    nc.vector.tensor_scalar(
                        out=sig_fg, in0=ps_pair[1], scalar1=1.0 / WG_SCALE,
                        scalar2=None, op0=ALU.mult,
                    )
                    nc.vector.tensor_add(
                        out=sig_fg, in0=sig_fg,
                        in1=bgr_sb[:, D + dh * 512 : D + (dh + 1) * 512],
                    )
                    nc.scalar.activation(out=sig_fg, in_=sig_fg, func=AF.Sigmoid)
                else:
                    nc.scalar.activation(
                        out=sig_in, in_=ps_pair[0], func=AF.Sigmoid,
                        scale=1.0 / WG_SCALE,
                    )
                    nc.scalar.activation(
                        out=sig_fg, in_=ps_pair[1], func=AF.Sigmoid,
                        scale=1.0 / WG_SCALE,
                    )

                m1 = g_p.tile([P, 512], FP32, tag="m1")
                nc.vector.tensor_mul(out=m1, in0=sig_in, in1=x_re)
                m2 = g_p.tile([P, 512], FP32, tag="m2")
                nc.gpsimd.tensor_mul(out=m2, in0=sig_fg, in1=ao_sb[:, sb, ds_])
                gt = g_p.tile([P, 512], FP32, tag="gt")
                nc.vector.tensor_add(out=gt, in0=m1, in1=m2)
                if gt_prev[0] is not None:
                    # previous block's write: its DVE add has landed, so this
                    # DMA doesn't hold the queue SEQ waiting
                    pt, prow, pds = gt_prev[0]
                    nc.sync.dma_start(out=gated_d[prow, pds], in_=pt)
                gt_prev[0] = (gt, slice(sb * P, (sb + 1) * P), ds_)

        pt, prow, pds = gt_prev[0]
        nc.sync.dma_start(out=gated_d[prow, pds], in_=pt)

    nc.compile()
    return nc


def host_inputs(x, w1, b1, w2, b2, ln_g, ln_b, wg, bg):
    """Fold LN affine params into w1/b1, precompute constants, cast weights."""
    x = np.asarray(x, np.float32)
    w1 = np.asarray(w1, np.float32)
    w2 = np.asarray(w2, np.float32)
    wg = np.asarray(wg, np.float32)
    ln_g = np.asarray(ln_g, np.float32)
    ln_b = np.asarray(ln_b, np.float32)
    b1 = np.asarray(b1, np.float32)

    w1g = (ln_g[:, None] * w1).astype(ml_dtypes.bfloat16)
    b1p = (b1 + ln_b @ w1).astype(np.float32)
    tri = np.triu(np.ones((P, P), np.float32))
    tri16 = np.triu(np.ones((16, 16), np.float32), k=1)
    eblk = np.zeros((P, NBLK, 16), np.float32)
    for i in range(NBLK):
        eblk[:, i, i] = 1.0
    esel = np.zeros((16, NBLK, P), np.float32)
    for i in range(NBLK):
        esel[i, i, :] = 1.0
    pos = np.arange(S, dtype=np.float64).reshape(NBLK, P).T  # [P, NBLK]
    invpos = (1.0 / (pos + 1.0)).astype(np.float32)

    # host-tile the (scaled) fp8 gating weights into the SBUF layout
    wg8 = (wg * WG_SCALE).astype(ml_dtypes.float8_e4m3)
    wg8t = np.ascontiguousarray(
        wg8.reshape(2 * KC, P, D2).transpose(1, 0, 2)
    )  # [128, 16, 2048]

    base = {
        "x": None,  # per-core
        "xbf": None,
        "w1g": w1g,
        "b1p": b1p,
        "w2": w2.astype(ml_dtypes.bfloat16),
        "wg8in0": np.ascontiguousarray(wg8t[:, :, 0:512]),
        "wg8in1": np.ascontiguousarray(wg8t[:, :, 512:1024]),
        "wg8fg0": np.ascontiguousarray(wg8t[:, :, D : D + 512]),
        "wg8fg1": np.ascontiguousarray(wg8t[:, :, D + 512 : D + 1024]),
        "tri": tri.astype(ml_dtypes.bfloat16),
        "tri16": tri16.astype(ml_dtypes.bfloat16),
        "eblk": eblk.astype(ml_dtypes.bfloat16),
        "esel": esel.astype(ml_dtypes.bfloat16),
        "invpos": invpos,
    }
    has_b2 = bool(np.any(b2))
    has_bg = bool(np.any(bg))
    if has_b2:
        base["b2"] = np.asarray(b2, np.float32)
    if has_bg:
        base["bg"] = np.asarray(bg, np.float32)
    return base, has_b2, has_bg


_prog_cache = {}


def kernel(x, w1, b1, w2, b2, ln_g, ln_b, wg, bg):
    x = np.asarray(x, np.float32)
    assert x.shape == (B, S, D), x.shape
    base, has_b2, has_bg = host_inputs(x, w1, b1, w2, b2, ln_g, ln_b, wg, bg)

    key = (has_b2, has_bg)
    if key not in _prog_cache:
        _prog_cache[key] = build_program(has_b2, has_bg)
    nc = _prog_cache[key]

    xbf = x.astype(ml_dtypes.bfloat16)
    in_maps = []
    for core in range(B):
        m = dict(base)
        m["x"] = np.ascontiguousarray(x[core])
        m["xbf"] = np.ascontiguousarray(xbf[core])
        in_maps.append(m)

    res = run_bass_kernel_spmd(nc, in_maps, core_ids=list(range(B)))
    gated = np.stack([res.results[c]["gated"] for c in range(B)])
    avg_out = np.stack([res.results[c]["avg_out"] for c in range(B)])
    return gated, avg_out
